# revision 21
# baseline (speedup 1.0000x reference)
"""Trainium2 Bass kernel for a 3-layer GIN-style GNN (gather + segment-sum +
MLP + BatchNorm + ReLU per layer, linear head).

Sharding: nodes (segment_sum destinations) are partitioned across the 8
NeuronCores; each core owns 6250 destination rows and all edges targeting
them.  Per layer, each core gathers source features for its edges from a
replicated fp16 copy of h (dma_gather), reduces them into per-destination
sums with one-hot matmuls on the tensor engine (PSUM accumulation), runs the
dense MLP on its shard, computes BatchNorm statistics locally and combines
them with a tiny AllReduce, and finally AllGathers the new h shards so every
core again holds the full feature table for the next layer's gather.
"""
import sys
sys.path.insert(0, '/opt/trn_rl_repo')

import numpy as np

N = 50000
E = 800000
D = 128
C = 8                    # cores
S = N // C               # 6250 destinations per core
WIN = 128                # destination window (PSUM tile width)
NW = (S + WIN - 1) // WIN  # 49 windows (last one 106 wide)
HALF = N // 2            # gather-table half size (int16 index range)
BN_EPS = 1e-5
import os as _os
DBG_NOCC = int(_os.environ.get("KV_NOCC", "0"))      # skip collectives
DBG_NOTR = int(_os.environ.get("KV_NOTR", "0"))      # skip PE-transpose path
DBG_NOGATHER = int(_os.environ.get("KV_NOGATHER", "0"))  # memset instead of gather
DBG_QSPREAD = int(_os.environ.get("KV_QSPREAD", "4"))    # SWDGE queues to spread over
GBUFS = int(_os.environ.get("KV_GBUFS", "8"))            # gather chunk buffers
# SWDGE descriptor carveout is dynamic_dma_scratch_size/16 = 1024 descriptors;
# a single dma_gather must stay under it or the Q7 waits forever.
CH = int(_os.environ.get("KV_CH", "8"))  # gather chunk, in 128-edge blocks
DMA_SCRATCH = int(_os.environ.get("KV_SCRATCH", str(max(16384, CH * 128 * 16))))
GB = 16                  # one-hot group, in 128-edge blocks
MC = 512                 # MLP free-dim chunk

_CACHE = {}


def _prep_edges(rows, cols):
    rows = np.asarray(rows, np.int64)
    cols = np.asarray(cols, np.int64)
    core = rows // S
    win = (rows % S) // WIN
    slot = (rows % S) % WIN
    half = cols // HALF
    idx = cols % HALF

    counts = np.zeros((C, 2, NW), np.int64)
    np.add.at(counts, (core, half, win), 1)
    nblk = np.maximum(1, -(-counts.max(axis=0) // 128))  # [2, NW]
    NB = nblk.sum(axis=1).astype(int)                    # blocks per half

    idxs = np.zeros((C, 2, int(NB.max()) * 128), np.int64)
    slots = np.full((C, 2, int(NB.max()) * 128), -1, np.int64)
    order = np.lexsort((win, half, core))
    ro_idx, ro_slot = idx[order], slot[order]
    pos = 0
    for c in range(C):
        for h in range(2):
            off = 0
            for w in range(NW):
                n = counts[c, h, w]
                idxs[c, h, off:off + n] = ro_idx[pos:pos + n]
                slots[c, h, off:off + n] = ro_slot[pos:pos + n]
                off += nblk[h, w] * 128
                pos += n
    assert pos == E
    return nblk, NB, idxs, slots


def _wrap_idxs(idx):
    """dma_gather index layout: logical i at [i%16, i//16], tiled to 128
    partitions (8 replicas for the 8 GPSIMD cores)."""
    arr = idx.reshape(-1, 16).T.astype(np.int16)
    return np.tile(arr, (8, 1)).copy()


def _build(nblk, NB, bout_val):
    import concourse.bacc as bacc
    import concourse.mybir as mybir
    import concourse.tile as tile

    f32, f16, i16 = mybir.dt.float32, mybir.dt.float16, mybir.dt.int16
    f32r = mybir.dt.float32r
    AF = mybir.ActivationFunctionType
    ALU = mybir.AluOpType

    nc = bacc.Bacc("TRN2", target_bir_lowering=False, debug=False, num_devices=C,
                   dynamic_dma_scratch_size=DMA_SCRATCH,
                   num_swdge_queues=max(1, DBG_QSPREAD))

    h0 = nc.dram_tensor("h0", [N, D], f16, kind="ExternalInput")
    idx_d = [nc.dram_tensor(f"idx{h}", [128, int(NB[h]) * 8], i16, kind="ExternalInput")
             for h in range(2)]
    slot_d = [nc.dram_tensor(f"slot{h}", [128, int(NB[h])], f32, kind="ExternalInput")
              for h in range(2)]
    iota_d = nc.dram_tensor("iota", [128, 128], f16, kind="ExternalInput")
    ident_d = nc.dram_tensor("ident", [128, 128], f16, kind="ExternalInput")
    wt_d = {}
    for li in range(3):
        for nm in ("w1", "w2"):
            wt_d[nm, li] = nc.dram_tensor(f"{nm}_{li}", [128, 128], f32, kind="ExternalInput")
        for nm in ("b1", "g1", "bt1", "b2", "g", "bt"):
            wt_d[nm, li] = nc.dram_tensor(f"{nm}_{li}", [128, 1], f32, kind="ExternalInput")
    wout_d = nc.dram_tensor("wout", [128, 1], f32, kind="ExternalInput")
    out_d = nc.dram_tensor("out", [1, S], f32, kind="ExternalOutput")

    h_full = [None,
              nc.dram_tensor("h_full1", [N, D], f16, addr_space="Shared"),
              nc.dram_tensor("h_full2", [N, D], f16, addr_space="Shared")]
    hsh = [None,
           nc.dram_tensor("hsh1", [S, D], f16),
           nc.dram_tensor("hsh2", [S, D], f16)]
    ar_in_d = [nc.dram_tensor(f"ar_in{k}", [128, 2], f32) for k in range(6)]
    ar_out_d = [nc.dram_tensor(f"ar_out{k}", [128, 2], f32, addr_space="Shared")
                for k in range(6)]

    with tile.TileContext(nc) as tc:
        with (
            tc.tile_pool(name="persist", bufs=1) as pp,
            tc.tile_pool(name="gpool", bufs=GBUFS) as gp,
            tc.tile_pool(name="ohpool", bufs=3) as ohp,
            tc.tile_pool(name="big", bufs=3) as bigp,
            tc.tile_pool(name="h16", bufs=2) as h16p,
            tc.tile_pool(name="small", bufs=4) as smp,
            tc.tile_pool(name="trs", bufs=4) as trp,
            tc.tile_pool(name="pwin", bufs=4, space="PSUM") as pwin,
            tc.tile_pool(name="pmlp", bufs=2, space="PSUM") as pmlp,
            tc.tile_pool(name="ptr", bufs=2, space="PSUM") as ptr,
        ):
            # --- persistent loads -------------------------------------------
            idx_t, slot_t = [], []
            for h in range(2):
                it = pp.tile([128, int(NB[h]) * 8], i16, tag=f"idx{h}")
                nc.sync.dma_start(it[:], idx_d[h][:])
                idx_t.append(it)
                st = pp.tile([128, int(NB[h])], f32, tag=f"slot{h}")
                nc.sync.dma_start(st[:], slot_d[h][:])
                slot_t.append(st)
            iota_t = pp.tile([128, 128], f16, tag="iota")
            nc.sync.dma_start(iota_t[:], iota_d[:])
            ident_t = pp.tile([128, 128], f16, tag="ident")
            nc.sync.dma_start(ident_t[:], ident_d[:])
            wt = {}
            for k, dram in wt_d.items():
                t = pp.tile(list(dram.shape), f32, tag=f"{k[0]}_{k[1]}")
                nc.sync.dma_start(t[:], dram[:])
                wt[k] = t
            wout_t = pp.tile([128, 1], f32, tag="wout")
            nc.sync.dma_start(wout_t[:], wout_d[:])
            eps_t = pp.tile([128, 1], f32, tag="eps")
            nc.vector.memset(eps_t[:], BN_EPS)
            bout_t = pp.tile([128, 1], f32, tag="bout")
            nc.vector.memset(bout_t[:], float(bout_val))

            ar_k = 0
            gq = [0]

            def bn_block(y_sb, gamma, beta, relu_out, relu_dtype):
                """bn stats over y_sb[:, :S] -> AllReduce -> scale/shift ->
                relu_out = relu(y*sc+sh) written as relu_dtype."""
                nonlocal ar_k
                nch = (S + MC - 1) // MC
                stats = smp.tile([128, nch, 6], f32, tag="stats")
                for k in range(nch):
                    sl = slice(k * MC, min((k + 1) * MC, S))
                    nc.vector.bn_stats(stats[:, k, :], y_sb[:, sl])
                ms = smp.tile([128, 2], f32, tag="ms")
                nc.vector.bn_aggr(ms[:], stats[:])
                # pack [mean, var + mean^2]
                ari = smp.tile([128, 2], f32, tag="ari")
                nc.vector.tensor_copy(ari[:, 0:1], ms[:, 0:1])
                sq = smp.tile([128, 1], f32, tag="sq")
                nc.vector.tensor_tensor(sq[:], ms[:, 0:1], ms[:, 0:1], ALU.mult)
                nc.vector.tensor_tensor(ari[:, 1:2], ms[:, 1:2], sq[:], ALU.add)
                nc.sync.dma_start(ar_in_d[ar_k][:], ari[:])
                aro = smp.tile([128, 2], f32, tag="aro")
                if DBG_NOCC:
                    nc.scalar.mul(aro[:], ari[:], float(C))
                else:
                    nc.gpsimd.collective_compute(
                        "AllReduce", ALU.add,
                        replica_groups=[list(range(C))],
                        ins=[ar_in_d[ar_k][:]],
                        outs=[ar_out_d[ar_k][:]],
                    )
                    nc.sync.dma_start(aro[:], ar_out_d[ar_k][:])
                ar_k += 1
                mean_g = smp.tile([128, 1], f32, tag="mean_g")
                nc.scalar.mul(mean_g[:], aro[:, 0:1], 1.0 / C)
                ex2 = smp.tile([128, 1], f32, tag="ex2")
                nc.scalar.mul(ex2[:], aro[:, 1:2], 1.0 / C)
                msq = smp.tile([128, 1], f32, tag="msq")
                nc.vector.tensor_tensor(msq[:], mean_g[:], mean_g[:], ALU.mult)
                var_g = smp.tile([128, 1], f32, tag="var_g")
                nc.vector.tensor_sub(var_g[:], ex2[:], msq[:])
                sqv = smp.tile([128, 1], f32, tag="sqv")
                nc.scalar.activation(sqv[:], var_g[:], AF.Sqrt, bias=eps_t[:])
                inv = smp.tile([128, 1], f32, tag="inv")
                nc.vector.reciprocal(inv[:], sqv[:])
                sc = smp.tile([128, 1], f32, tag="sc")
                nc.vector.tensor_tensor(sc[:], gamma[:], inv[:], ALU.mult)
                tmp = smp.tile([128, 1], f32, tag="tmp")
                nc.vector.tensor_tensor(tmp[:], mean_g[:], sc[:], ALU.mult)
                sh = smp.tile([128, 1], f32, tag="sh")
                nc.vector.tensor_sub(sh[:], beta[:], tmp[:])
                # apply in two halves for a little pipelining
                mid = (S // 2) // MC * MC
                for sl in (slice(0, mid), slice(mid, S)):
                    nc.scalar.activation(relu_out[:, sl], y_sb[:, sl], AF.Relu,
                                         bias=sh[:], scale=sc[:])

            for li in range(3):
                table = h0 if li == 0 else h_full[li]

                # --- gather + pooling -----------------------------------------
                poolT = bigp.tile([128, S], f32, tag="big")
                for h in range(2):
                    nbh = int(NB[h])
                    tbl_ap = table[h * HALF:(h + 1) * HALF, :]
                    # chunked gathers
                    chunks = []      # (start_blk, tile, nblk)
                    for cs in range(0, nbh, CH):
                        cb = min(CH, nbh - cs)
                        g_t = gp.tile([128, CH, D], f16, tag="g")
                        if DBG_NOGATHER:
                            nc.vector.memset(g_t[:, :cb, :], 0.25)
                        else:
                            nc.gpsimd.dma_gather(
                                g_t[:, :cb, :], tbl_ap,
                                idx_t[h][:, cs * 8:(cs + cb) * 8],
                                cb * 128, cb * 128, D,
                                queue_num=(gq[0] % DBG_QSPREAD),
                                single_packet=bool(int(_os.environ.get("KV_SP", "0"))),
                            )
                            gq[0] += 1
                        chunks.append((cs, g_t, cb))
                    # one-hot groups: per-block tensor_scalar (4x DVE mode)
                    groups = []
                    for gs in range(0, nbh, GB):
                        gb = min(GB, nbh - gs)
                        oh_t = ohp.tile([128, GB, 128], f16, tag="oh")
                        for b in range(gb):
                            nc.vector.tensor_scalar(
                                oh_t[:, b, :], iota_t[:],
                                slot_t[h][:, gs + b:gs + b + 1], None,
                                ALU.is_equal)
                        groups.append((gs, oh_t, gb))
                    # per-window matmul accumulate + evict
                    blk = 0
                    ci = gi = 0
                    for w in range(NW):
                        ps = pwin.tile([128, 128], f32, tag="pwin")
                        nb = int(nblk[h, w])
                        for b in range(nb):
                            if blk >= chunks[ci][0] + chunks[ci][2]:
                                ci += 1
                            if blk >= groups[gi][0] + groups[gi][2]:
                                gi += 1
                            g_t = chunks[ci][1]
                            oh_t = groups[gi][1]
                            nc.tensor.matmul(
                                ps[:], g_t[:, blk - chunks[ci][0], :],
                                oh_t[:, blk - groups[gi][0], :],
                                start=(b == 0), stop=(b == nb - 1),
                            )
                            blk += 1
                        wl = min(WIN, S - w * WIN)
                        dst = poolT[:, w * WIN: w * WIN + wl]
                        if h == 0:
                            nc.scalar.copy(dst, ps[:, :wl])
                        else:
                            nc.vector.tensor_add(dst, ps[:, :wl], dst)

                # --- MLP lin1 + BN1 + relu ------------------------------------
                y1 = bigp.tile([128, S], f32, tag="big")
                for k in range((S + MC - 1) // MC):
                    sl = slice(k * MC, min((k + 1) * MC, S))
                    cw = sl.stop - sl.start
                    ps = pmlp.tile([128, MC], f32, tag="pmlp")
                    nc.tensor.matmul(ps[:, :cw], wt["w1", li][:],
                                     poolT[:, sl], start=True, stop=True)
                    nc.scalar.activation(y1[:, sl], ps[:, :cw], AF.Identity,
                                         bias=wt["b1", li][:])
                h1 = bigp.tile([128, S], f32, tag="big")
                bn_block(y1, wt["g1", li], wt["bt1", li], h1, f32)

                # --- lin2 + BN2 + relu ----------------------------------------
                y2 = bigp.tile([128, S], f32, tag="big")
                for k in range((S + MC - 1) // MC):
                    sl = slice(k * MC, min((k + 1) * MC, S))
                    cw = sl.stop - sl.start
                    ps = pmlp.tile([128, MC], f32, tag="pmlp")
                    nc.tensor.matmul(ps[:, :cw], wt["w2", li][:],
                                     h1[:, sl], start=True, stop=True)
                    nc.scalar.activation(y2[:, sl], ps[:, :cw], AF.Identity,
                                         bias=wt["b2", li][:])

                if li < 2:
                    hT = h16p.tile([128, S], f16, tag="h16")
                    bn_block(y2, wt["g", li], wt["bt", li], hT, f16)
                    # transpose windows out to hsh, then AllGather
                    for w in ([] if DBG_NOTR else range(NW)):
                        wl = min(WIN, S - w * WIN)
                        pt = ptr.tile([128, 128], f16, tag="ptr")
                        nc.tensor.transpose(pt[:wl, :], hT[:, w * WIN:w * WIN + wl],
                                            ident_t[:])
                        ts = trp.tile([128, 128], f16, tag="trs")
                        nc.scalar.copy(ts[:wl, :], pt[:wl, :])
                        nc.sync.dma_start(hsh[li + 1][w * WIN:w * WIN + wl, :],
                                          ts[:wl, :])
                    if DBG_NOCC:
                        nc.sync.dma_start(h_full[li + 1][0:S, :], hsh[li + 1][:])
                    else:
                        nc.gpsimd.collective_compute(
                            "AllGather", mybir.AluOpType.bypass,
                            replica_groups=[list(range(C))],
                            ins=[hsh[li + 1][:]],
                            outs=[h_full[li + 1][:]],
                        )
                else:
                    hT = bigp.tile([128, S], f32, tag="big")
                    bn_block(y2, wt["g", li], wt["bt", li], hT, f32)
                    out_sb = pp.tile([1, S], f32, tag="out_sb")
                    for k in range((S + MC - 1) // MC):
                        sl = slice(k * MC, min((k + 1) * MC, S))
                        cw = sl.stop - sl.start
                        ps = pmlp.tile([128, MC], f32, tag="pmlp")
                        nc.tensor.matmul(ps[:1, :cw], wout_t[:],
                                         hT[:, sl], start=True, stop=True)
                        nc.scalar.activation(out_sb[:, sl], ps[:1, :cw], AF.Identity,
                                             bias=bout_t[:1, :])
                    nc.sync.dma_start(out_d[:], out_sb[:])
    nc.compile()
    return nc


def kernel(seq1, rows, cols, params):
    from concourse.bass_utils import run_bass_kernel_spmd

    seq1 = np.asarray(seq1, np.float32)
    rows_np = np.asarray(rows)
    cols_np = np.asarray(cols)
    nblk, NB, idxs, slots = _prep_edges(rows_np, cols_np)
    bout_val = float(np.asarray(params["bout"]).reshape(-1)[0])

    key = (tuple(nblk.reshape(-1).tolist()), bout_val,
           CH, DBG_NOCC, DBG_NOTR, DBG_NOGATHER, DBG_QSPREAD,
           _os.environ.get('KV_SP', '0'), GBUFS)
    if key not in _CACHE:
        _CACHE[key] = _build(nblk, NB, bout_val)
    nc = _CACHE[key]

    h0 = seq1.astype(np.float16)
    iota = np.tile(np.arange(128, dtype=np.float16), (128, 1)).copy()
    ident = np.eye(128, dtype=np.float16)

    layers = params["layers"]
    common = {"h0": h0, "iota": iota, "ident": ident,
              "wout": np.asarray(params["Wout"], np.float32).reshape(128, 1)}
    for li, lp in enumerate(layers):
        common[f"w1_{li}"] = np.asarray(lp["W1"], np.float32)
        common[f"w2_{li}"] = np.asarray(lp["W2"], np.float32)
        for nm, src in (("b1", "b1"), ("g1", "g1"), ("bt1", "bt1"),
                        ("b2", "b2"), ("g", "g"), ("bt", "bt")):
            common[f"{nm}_{li}"] = np.asarray(lp[src], np.float32).reshape(128, 1)

    in_maps = []
    for c in range(C):
        m = dict(common)
        for h in range(2):
            n = int(NB[h]) * 128
            m[f"idx{h}"] = _wrap_idxs(idxs[c, h, :n])
            m[f"slot{h}"] = slots[c, h, :n].reshape(int(NB[h]), 128).T.astype(np.float32).copy()
        in_maps.append(m)

    import os
    global LAST_RESULT
    if bool(int(os.environ.get("KERNEL_SIM", "0"))):
        from concourse.bass_interp import MultiCoreSim
        sim = MultiCoreSim(nc, num_cores=C, trace=False)
        for c in range(C):
            for k, v in in_maps[c].items():
                sim.cores[c].tensor(k)[:] = v
        sim.simulate(check_with_hw=False, trace_hw=False)
        out = np.concatenate([np.array(sim.cores[c].mem_tensor("out")).reshape(S)
                              for c in range(C)])
        LAST_RESULT = None
        return out.reshape(N, 1).astype(np.float32)
    trace = bool(int(os.environ.get("KERNEL_TRACE", "0")))
    res = run_bass_kernel_spmd(nc, in_maps, core_ids=list(range(C)), trace=trace)
    LAST_RESULT = res
    out = np.concatenate([res.results[c]["out"].reshape(S) for c in range(C)])
    return out.reshape(N, 1).astype(np.float32)


LAST_RESULT = None


# revision 23
# speedup vs baseline: 1.3131x; 1.3131x over previous
"""Trainium2 Bass kernel for a 3-layer GIN-style GNN (gather + segment-sum +
MLP + BatchNorm + ReLU per layer, linear head).

Sharding: nodes (segment_sum destinations) are partitioned across the 8
NeuronCores; each core owns 6250 destination rows and all edges targeting
them.  Per layer, each core gathers source features for its edges from a
replicated fp16 copy of h (dma_gather), reduces them into per-destination
sums with one-hot matmuls on the tensor engine (PSUM accumulation), runs the
dense MLP on its shard, computes BatchNorm statistics locally and combines
them with a tiny AllReduce, and finally AllGathers the new h shards so every
core again holds the full feature table for the next layer's gather.
"""
import sys
sys.path.insert(0, '/opt/trn_rl_repo')

import numpy as np

N = 50000
E = 800000
D = 128
C = 8                    # cores
S = N // C               # 6250 destinations per core
WIN = 128                # destination window (PSUM tile width)
NW = (S + WIN - 1) // WIN  # 49 windows (last one 106 wide)
HALF = N // 2            # gather-table half size (int16 index range)
BN_EPS = 1e-5
import os as _os
DBG_NOCC = int(_os.environ.get("KV_NOCC", "0"))      # skip collectives
DBG_NOTR = int(_os.environ.get("KV_NOTR", "0"))      # skip PE-transpose path
DBG_NOGATHER = int(_os.environ.get("KV_NOGATHER", "0"))  # memset instead of gather
DBG_QSPREAD = int(_os.environ.get("KV_QSPREAD", "4"))    # SWDGE queues to spread over
GBUFS = int(_os.environ.get("KV_GBUFS", "4"))            # gather chunk buffers
# SWDGE descriptor carveout is dynamic_dma_scratch_size/16 = 1024 descriptors;
# a single dma_gather must stay under it or the Q7 waits forever.
CH = int(_os.environ.get("KV_CH", "8"))  # gather chunk, in 128-edge blocks
DMA_SCRATCH = int(_os.environ.get("KV_SCRATCH", str(max(16384, CH * 128 * 16))))
GB = 16                  # one-hot group, in 128-edge blocks
MC = 512                 # MLP free-dim chunk

_CACHE = {}


def _prep_edges(rows, cols):
    rows = np.asarray(rows, np.int64)
    cols = np.asarray(cols, np.int64)
    core = rows // S
    win = (rows % S) // WIN
    slot = (rows % S) % WIN
    half = cols // HALF
    idx = cols % HALF

    counts = np.zeros((C, 2, NW), np.int64)
    np.add.at(counts, (core, half, win), 1)
    nblk = np.maximum(1, -(-counts.max(axis=0) // 128))  # [2, NW]
    NB = nblk.sum(axis=1).astype(int)                    # blocks per half

    idxs = np.zeros((C, 2, int(NB.max()) * 128), np.int64)
    slots = np.full((C, 2, int(NB.max()) * 128), -1, np.int64)
    order = np.lexsort((win, half, core))
    ro_idx, ro_slot = idx[order], slot[order]
    pos = 0
    for c in range(C):
        for h in range(2):
            off = 0
            for w in range(NW):
                n = counts[c, h, w]
                idxs[c, h, off:off + n] = ro_idx[pos:pos + n]
                slots[c, h, off:off + n] = ro_slot[pos:pos + n]
                off += nblk[h, w] * 128
                pos += n
    assert pos == E
    return nblk, NB, idxs, slots


def _wrap_idxs(idx):
    """dma_gather index layout: logical i at [i%16, i//16], tiled to 128
    partitions (8 replicas for the 8 GPSIMD cores)."""
    arr = idx.reshape(-1, 16).T.astype(np.int16)
    return np.tile(arr, (8, 1)).copy()


def _build(nblk, NB, bout_val):
    import concourse.bacc as bacc
    import concourse.mybir as mybir
    import concourse.tile as tile

    f32, f16, i16 = mybir.dt.float32, mybir.dt.float16, mybir.dt.int16
    f32r = mybir.dt.float32r
    AF = mybir.ActivationFunctionType
    ALU = mybir.AluOpType

    nc = bacc.Bacc("TRN2", target_bir_lowering=False, debug=False, num_devices=C,
                   dynamic_dma_scratch_size=DMA_SCRATCH,
                   num_swdge_queues=max(1, DBG_QSPREAD))

    h0 = nc.dram_tensor("h0", [N, D], f16, kind="ExternalInput")
    idx_d = [nc.dram_tensor(f"idx{h}", [128, int(NB[h]) * 8], i16, kind="ExternalInput")
             for h in range(2)]
    slot_d = [nc.dram_tensor(f"slot{h}", [128, int(NB[h])], f32, kind="ExternalInput")
              for h in range(2)]
    iota_d = nc.dram_tensor("iota", [128, 128], f16, kind="ExternalInput")
    ident_d = nc.dram_tensor("ident", [128, 128], f16, kind="ExternalInput")
    wt_d = {}
    for li in range(3):
        for nm in ("w1", "w2"):
            wt_d[nm, li] = nc.dram_tensor(f"{nm}_{li}", [128, 128], f32, kind="ExternalInput")
        for nm in ("b1", "g1", "bt1", "b2", "g", "bt"):
            wt_d[nm, li] = nc.dram_tensor(f"{nm}_{li}", [128, 1], f32, kind="ExternalInput")
    wout_d = nc.dram_tensor("wout", [128, 1], f32, kind="ExternalInput")
    out_d = nc.dram_tensor("out", [1, S], f32, kind="ExternalOutput")

    h_full = [None,
              nc.dram_tensor("h_full1", [N, D], f16, addr_space="Shared"),
              nc.dram_tensor("h_full2", [N, D], f16, addr_space="Shared")]
    hsh = [None,
           nc.dram_tensor("hsh1", [S, D], f16),
           nc.dram_tensor("hsh2", [S, D], f16)]
    ar_in_d = [nc.dram_tensor(f"ar_in{k}", [128, 2], f32) for k in range(6)]
    ar_out_d = [nc.dram_tensor(f"ar_out{k}", [128, 2], f32, addr_space="Shared")
                for k in range(6)]

    with tile.TileContext(nc) as tc:
        with (
            tc.tile_pool(name="persist", bufs=1) as pp,
            tc.tile_pool(name="gpool", bufs=GBUFS) as gp,
            tc.tile_pool(name="ohpool", bufs=3) as ohp,
            tc.tile_pool(name="big", bufs=3) as bigp,
            tc.tile_pool(name="h16", bufs=2) as h16p,
            tc.tile_pool(name="small", bufs=4) as smp,
            tc.tile_pool(name="trs", bufs=4) as trp,
            tc.tile_pool(name="pwin", bufs=4, space="PSUM") as pwin,
            tc.tile_pool(name="pmlp", bufs=2, space="PSUM") as pmlp,
            tc.tile_pool(name="ptr", bufs=2, space="PSUM") as ptr,
        ):
            # --- persistent loads -------------------------------------------
            idx_t, slot_t = [], []
            for h in range(2):
                it = pp.tile([128, int(NB[h]) * 8], i16, tag=f"idx{h}")
                nc.sync.dma_start(it[:], idx_d[h][:])
                idx_t.append(it)
                st = pp.tile([128, int(NB[h])], f32, tag=f"slot{h}")
                nc.sync.dma_start(st[:], slot_d[h][:])
                slot_t.append(st)
            iota_t = pp.tile([128, 128], f16, tag="iota")
            nc.sync.dma_start(iota_t[:], iota_d[:])
            ident_t = pp.tile([128, 128], f16, tag="ident")
            nc.sync.dma_start(ident_t[:], ident_d[:])
            wt = {}
            for k, dram in wt_d.items():
                t = pp.tile(list(dram.shape), f32, tag=f"{k[0]}_{k[1]}")
                nc.sync.dma_start(t[:], dram[:])
                wt[k] = t
            wout_t = pp.tile([128, 1], f32, tag="wout")
            nc.sync.dma_start(wout_t[:], wout_d[:])
            eps_t = pp.tile([128, 1], f32, tag="eps")
            nc.vector.memset(eps_t[:], BN_EPS)
            bout_t = pp.tile([128, 1], f32, tag="bout")
            nc.vector.memset(bout_t[:], float(bout_val))

            ar_k = 0
            gq = [0]

            def bn_block(y_sb, gamma, beta, relu_out, relu_dtype):
                """bn stats over y_sb[:, :S] -> AllReduce -> scale/shift ->
                relu_out = relu(y*sc+sh) written as relu_dtype."""
                nonlocal ar_k
                nch = (S + MC - 1) // MC
                stats = smp.tile([128, nch, 6], f32, tag="stats")
                for k in range(nch):
                    sl = slice(k * MC, min((k + 1) * MC, S))
                    nc.vector.bn_stats(stats[:, k, :], y_sb[:, sl])
                ms = smp.tile([128, 2], f32, tag="ms")
                nc.vector.bn_aggr(ms[:], stats[:])
                # pack [mean, var + mean^2]
                ari = smp.tile([128, 2], f32, tag="ari")
                nc.vector.tensor_copy(ari[:, 0:1], ms[:, 0:1])
                sq = smp.tile([128, 1], f32, tag="sq")
                nc.vector.tensor_tensor(sq[:], ms[:, 0:1], ms[:, 0:1], ALU.mult)
                nc.vector.tensor_tensor(ari[:, 1:2], ms[:, 1:2], sq[:], ALU.add)
                nc.sync.dma_start(ar_in_d[ar_k][:], ari[:])
                aro = smp.tile([128, 2], f32, tag="aro")
                if DBG_NOCC:
                    nc.scalar.mul(aro[:], ari[:], float(C))
                else:
                    nc.gpsimd.collective_compute(
                        "AllReduce", ALU.add,
                        replica_groups=[list(range(C))],
                        ins=[ar_in_d[ar_k][:]],
                        outs=[ar_out_d[ar_k][:]],
                    )
                    nc.sync.dma_start(aro[:], ar_out_d[ar_k][:])
                ar_k += 1
                mean_g = smp.tile([128, 1], f32, tag="mean_g")
                nc.scalar.mul(mean_g[:], aro[:, 0:1], 1.0 / C)
                ex2 = smp.tile([128, 1], f32, tag="ex2")
                nc.scalar.mul(ex2[:], aro[:, 1:2], 1.0 / C)
                msq = smp.tile([128, 1], f32, tag="msq")
                nc.vector.tensor_tensor(msq[:], mean_g[:], mean_g[:], ALU.mult)
                var_g = smp.tile([128, 1], f32, tag="var_g")
                nc.vector.tensor_sub(var_g[:], ex2[:], msq[:])
                sqv = smp.tile([128, 1], f32, tag="sqv")
                nc.scalar.activation(sqv[:], var_g[:], AF.Sqrt, bias=eps_t[:])
                inv = smp.tile([128, 1], f32, tag="inv")
                nc.vector.reciprocal(inv[:], sqv[:])
                sc = smp.tile([128, 1], f32, tag="sc")
                nc.vector.tensor_tensor(sc[:], gamma[:], inv[:], ALU.mult)
                tmp = smp.tile([128, 1], f32, tag="tmp")
                nc.vector.tensor_tensor(tmp[:], mean_g[:], sc[:], ALU.mult)
                sh = smp.tile([128, 1], f32, tag="sh")
                nc.vector.tensor_sub(sh[:], beta[:], tmp[:])
                # apply in two halves for a little pipelining
                mid = (S // 2) // MC * MC
                for sl in (slice(0, mid), slice(mid, S)):
                    nc.scalar.activation(relu_out[:, sl], y_sb[:, sl], AF.Relu,
                                         bias=sh[:], scale=sc[:])

            for li in range(3):
                table = h0 if li == 0 else h_full[li]

                # --- gather + pooling -----------------------------------------
                poolT = bigp.tile([128, S], f32, tag="big")
                for h in range(2):
                    nbh = int(NB[h])
                    tbl_ap = table[h * HALF:(h + 1) * HALF, :]
                    # chunked gathers
                    chunks = []      # (start_blk, tile, nblk)
                    for cs in range(0, nbh, CH):
                        cb = min(CH, nbh - cs)
                        g_t = gp.tile([128, CH, D], f16, tag="g")
                        if DBG_NOGATHER == 2:
                            nc.sync.dma_start(
                                g_t[:, :cb, :],
                                tbl_ap[cs * 128 % (HALF - cb * 128):
                                       cs * 128 % (HALF - cb * 128) + cb * 128, :]
                                .rearrange("(b p) d -> p b d", p=128))
                        elif DBG_NOGATHER:
                            nc.vector.memset(g_t[:, :cb, :], 0.25)
                        else:
                            nc.gpsimd.dma_gather(
                                g_t[:, :cb, :], tbl_ap,
                                idx_t[h][:, cs * 8:(cs + cb) * 8],
                                cb * 128, cb * 128, D,
                                queue_num=(gq[0] % DBG_QSPREAD),
                                single_packet=bool(int(_os.environ.get("KV_SP", "0"))),
                            )
                            gq[0] += 1
                        chunks.append((cs, g_t, cb))
                    # one-hot groups: per-block tensor_scalar (4x DVE mode)
                    groups = []
                    for gs in range(0, nbh, GB):
                        gb = min(GB, nbh - gs)
                        oh_t = ohp.tile([128, GB, 128], f16, tag="oh")
                        for b in range(gb):
                            nc.vector.tensor_scalar(
                                oh_t[:, b, :], iota_t[:],
                                slot_t[h][:, gs + b:gs + b + 1], None,
                                ALU.is_equal)
                        groups.append((gs, oh_t, gb))
                    # per-window matmul accumulate + evict
                    blk = 0
                    ci = gi = 0
                    for w in range(NW):
                        ps = pwin.tile([128, 128], f32, tag="pwin")
                        nb = int(nblk[h, w])
                        for b in range(nb):
                            if blk >= chunks[ci][0] + chunks[ci][2]:
                                ci += 1
                            if blk >= groups[gi][0] + groups[gi][2]:
                                gi += 1
                            g_t = chunks[ci][1]
                            oh_t = groups[gi][1]
                            nc.tensor.matmul(
                                ps[:], g_t[:, blk - chunks[ci][0], :],
                                oh_t[:, blk - groups[gi][0], :],
                                start=(b == 0), stop=(b == nb - 1),
                            )
                            blk += 1
                        wl = min(WIN, S - w * WIN)
                        dst = poolT[:, w * WIN: w * WIN + wl]
                        if h == 0:
                            nc.scalar.copy(dst, ps[:, :wl])
                        else:
                            nc.vector.tensor_add(dst, ps[:, :wl], dst)

                # --- MLP lin1 + BN1 + relu ------------------------------------
                y1 = bigp.tile([128, S], f32, tag="big")
                for k in range((S + MC - 1) // MC):
                    sl = slice(k * MC, min((k + 1) * MC, S))
                    cw = sl.stop - sl.start
                    ps = pmlp.tile([128, MC], f32, tag="pmlp")
                    nc.tensor.matmul(ps[:, :cw], wt["w1", li][:],
                                     poolT[:, sl], start=True, stop=True)
                    nc.scalar.activation(y1[:, sl], ps[:, :cw], AF.Identity,
                                         bias=wt["b1", li][:])
                h1 = bigp.tile([128, S], f32, tag="big")
                bn_block(y1, wt["g1", li], wt["bt1", li], h1, f32)

                # --- lin2 + BN2 + relu ----------------------------------------
                y2 = bigp.tile([128, S], f32, tag="big")
                for k in range((S + MC - 1) // MC):
                    sl = slice(k * MC, min((k + 1) * MC, S))
                    cw = sl.stop - sl.start
                    ps = pmlp.tile([128, MC], f32, tag="pmlp")
                    nc.tensor.matmul(ps[:, :cw], wt["w2", li][:],
                                     h1[:, sl], start=True, stop=True)
                    nc.scalar.activation(y2[:, sl], ps[:, :cw], AF.Identity,
                                         bias=wt["b2", li][:])

                if li < 2:
                    hT = h16p.tile([128, S], f16, tag="h16")
                    bn_block(y2, wt["g", li], wt["bt", li], hT, f16)
                    # transpose windows out to hsh, then AllGather
                    for w in ([] if DBG_NOTR else range(NW)):
                        wl = min(WIN, S - w * WIN)
                        pt = ptr.tile([128, 128], f16, tag="ptr")
                        nc.tensor.transpose(pt[:wl, :], hT[:, w * WIN:w * WIN + wl],
                                            ident_t[:])
                        ts = trp.tile([128, 128], f16, tag="trs")
                        nc.scalar.copy(ts[:wl, :], pt[:wl, :])
                        nc.sync.dma_start(hsh[li + 1][w * WIN:w * WIN + wl, :],
                                          ts[:wl, :])
                    if DBG_NOCC:
                        nc.sync.dma_start(h_full[li + 1][0:S, :], hsh[li + 1][:])
                    else:
                        nc.gpsimd.collective_compute(
                            "AllGather", mybir.AluOpType.bypass,
                            replica_groups=[list(range(C))],
                            ins=[hsh[li + 1][:]],
                            outs=[h_full[li + 1][:]],
                        )
                else:
                    hT = bigp.tile([128, S], f32, tag="big")
                    bn_block(y2, wt["g", li], wt["bt", li], hT, f32)
                    out_sb = pp.tile([1, S], f32, tag="out_sb")
                    for k in range((S + MC - 1) // MC):
                        sl = slice(k * MC, min((k + 1) * MC, S))
                        cw = sl.stop - sl.start
                        ps = pmlp.tile([128, MC], f32, tag="pmlp")
                        nc.tensor.matmul(ps[:1, :cw], wout_t[:],
                                         hT[:, sl], start=True, stop=True)
                        nc.scalar.activation(out_sb[:, sl], ps[:1, :cw], AF.Identity,
                                             bias=bout_t[:1, :])
                    nc.sync.dma_start(out_d[:], out_sb[:])
    nc.compile()
    return nc


def kernel(seq1, rows, cols, params):
    from concourse.bass_utils import run_bass_kernel_spmd

    seq1 = np.asarray(seq1, np.float32)
    rows_np = np.asarray(rows)
    cols_np = np.asarray(cols)
    nblk, NB, idxs, slots = _prep_edges(rows_np, cols_np)
    bout_val = float(np.asarray(params["bout"]).reshape(-1)[0])

    key = (tuple(nblk.reshape(-1).tolist()), bout_val,
           CH, DBG_NOCC, DBG_NOTR, DBG_NOGATHER, DBG_QSPREAD,
           _os.environ.get('KV_SP', '0'), GBUFS)
    if key not in _CACHE:
        _CACHE[key] = _build(nblk, NB, bout_val)
    nc = _CACHE[key]

    h0 = seq1.astype(np.float16)
    iota = np.tile(np.arange(128, dtype=np.float16), (128, 1)).copy()
    ident = np.eye(128, dtype=np.float16)

    layers = params["layers"]
    common = {"h0": h0, "iota": iota, "ident": ident,
              "wout": np.asarray(params["Wout"], np.float32).reshape(128, 1)}
    for li, lp in enumerate(layers):
        common[f"w1_{li}"] = np.asarray(lp["W1"], np.float32)
        common[f"w2_{li}"] = np.asarray(lp["W2"], np.float32)
        for nm, src in (("b1", "b1"), ("g1", "g1"), ("bt1", "bt1"),
                        ("b2", "b2"), ("g", "g"), ("bt", "bt")):
            common[f"{nm}_{li}"] = np.asarray(lp[src], np.float32).reshape(128, 1)

    in_maps = []
    for c in range(C):
        m = dict(common)
        for h in range(2):
            n = int(NB[h]) * 128
            m[f"idx{h}"] = _wrap_idxs(idxs[c, h, :n])
            m[f"slot{h}"] = slots[c, h, :n].reshape(int(NB[h]), 128).T.astype(np.float32).copy()
        in_maps.append(m)

    import os
    global LAST_RESULT
    if bool(int(os.environ.get("KERNEL_SIM", "0"))):
        from concourse.bass_interp import MultiCoreSim
        sim = MultiCoreSim(nc, num_cores=C, trace=False)
        for c in range(C):
            for k, v in in_maps[c].items():
                sim.cores[c].tensor(k)[:] = v
        sim.simulate(check_with_hw=False, trace_hw=False)
        out = np.concatenate([np.array(sim.cores[c].mem_tensor("out")).reshape(S)
                              for c in range(C)])
        LAST_RESULT = None
        return out.reshape(N, 1).astype(np.float32)
    trace = bool(int(os.environ.get("KERNEL_TRACE", "0")))
    res = run_bass_kernel_spmd(nc, in_maps, core_ids=list(range(C)), trace=trace)
    LAST_RESULT = res
    out = np.concatenate([res.results[c]["out"].reshape(S) for c in range(C)])
    return out.reshape(N, 1).astype(np.float32)


LAST_RESULT = None
